# revision 1
# baseline (speedup 1.0000x reference)
"""MoE router gate (group-limited top-k) on 8 Trainium2 NeuronCores.

reference math (per token row of x [T=16384, D=4096], W [E=256, D]):
  logits = x @ W.T                      [T, 256]
  scores = softmax(logits)
  group (8 groups of 32) scores = max of scores per group
  keep top-4 groups, mask rest to -inf
  top-8 experts over masked scores -> indices
  weights = gathered softmax scores, renormalized over the 8 (+1e-9 in denom)

Sharding: data-parallel over tokens, 2048 tokens/core; W.T replicated.

GEMM strategy (the kernel is PE-bound; fp32 matmul costs 4 cyc/row, fp16
matmul 1 cyc/row): emulate the fp32 GEMM with three fp16 passes:

    logits = xh @ wh  +  (xl @ wh + xh @ wd) / 2048

  xh = fp16(x)                     (11-bit mantissa)
  xl = fp16((x - xh) * 2048)       (11 more bits; the 2^11 scale keeps the
                                    residual in fp16 normal range)
  wh = fp16(W.T)
  wd = fp16((W.T - wh) * 2048)

Products are exact in the PE (11+11-bit mantissas fit fp32), accumulation
is fp32 in PSUM: pass A accumulates in PSUM bank A, passes B+C (the 2^11-
scaled terms) share PSUM bank B, and the DVE combines A + B/2048. The only
error is the dropped (x-xh)@(W-wh)/2048^2 cross term and the 22-bit input
truncation: ~2e-7 rms on unit-scale logits — same quality as the fp32 PE
path (which matched the CPU reference's top-k exactly). Host prepares the
split tensors (cheap numpy casts), with xh/xl interleaved in one fp16
tensor x2 [T, 2*4096] so each 128-token tile needs a single DMA.

The x transpose (contraction dim must land on SBUF partitions) is done by
the DMA XBAR (dma_start_transpose, 16x128 tiles, ~14ns/tile), NOT the PE:
verified on HW that out[p, k, t] = in[t, k*128 + p], i.e. the same
d = k*128+p chunk convention the W load uses.

Per-core kernel, for each 128-token tile:
  - one XBAR-transpose of the x2 block [128t, 8192] -> [128d, 64k, 128t]
    (col-chunks 0..31 = xh d-chunks, 32..63 = xl d-chunks)
  - 96 fp16 matmuls (3 passes x 32 K-chunks) at 1 cyc/row accumulate
    [128t, 256e] into two PSUM tiles; DVE combines to fp32 logits
  - selection on raw logits (softmax is monotone per row):
      group maxes via 3D tensor_reduce, group top-4 threshold via DVE max
      (sorted top-8), additive -1e30 mask, DVE max + max_index for expert
      top-8 values/indices
  - weights = exp(v - M) / (sum8 + 1e-9 * Z), Z = full-row exp sum, via ACT
    activation(Exp, bias=-M, accum_out)

DMA queue placement: XBAR transposes own the SP HWDGE queue, weight loads
use the ACT HWDGE queue, output stores go through Pool SWDGE — so no DMA
ever head-of-line blocks the x-tile prefetch.
"""

import numpy as np

from concourse import bass, mybir
from concourse.bacc import Bacc
from concourse.tile import TileContext
from concourse.bass_utils import run_bass_kernel_spmd

TOKENS = 16384
DIM = 4096
E = 256
TOPK = 8
G = 8
GSZ = E // G  # 32
NL = 4  # groups kept
N_CORES = 8
TPC = TOKENS // N_CORES  # 2048 tokens per core
NT = TPC // 128  # 16 token tiles per core
KC = DIM // 128  # 32 contraction chunks
NEG_BIG = -1.0e30
LOSCALE = 2048.0  # 2^11
REPEAT = 1  # bench-only: replicate the tile loop on device

_CACHE = {}

f16 = mybir.dt.float16
f32 = mybir.dt.float32


def _build_program(repeat=None, loop_n=None):
    if repeat is None:
        repeat = REPEAT
    nc = Bacc()
    x2_ext = nc.declare_dram_parameter("x2", [TPC, 2 * DIM], f16, isOutput=False)
    wh_ext = nc.declare_dram_parameter("wh", [DIM, E], f16, isOutput=False)
    wd_ext = nc.declare_dram_parameter("wd", [DIM, E], f16, isOutput=False)
    w_out = nc.declare_dram_parameter(
        "weights", [TPC, TOPK], mybir.dt.float32, isOutput=True
    )
    i_out = nc.declare_dram_parameter(
        "indices", [TPC, TOPK], mybir.dt.int32, isOutput=True
    )

    with TileContext(nc) as tc:
        with (
            tc.tile_pool(name="const", bufs=1) as const_pool,
            tc.tile_pool(name="xt", bufs=4) as xt_pool,
            tc.tile_pool(name="plg", bufs=2, space="PSUM") as plg_pool,
            tc.tile_pool(name="mid", bufs=3) as mid_pool,
            tc.tile_pool(name="small", bufs=3) as small_pool,
        ):
            # W.T resident in SBUF: chunk k occupies columns [k*E, (k+1)*E),
            # partitions = contraction dim d within chunk (d = k*128 + p).
            # Loaded on the ACT HWDGE queue (XBAR transposes own the SP
            # queue); halves so pass A of tile 0 can start sooner.
            wh_sb = const_pool.tile([128, KC * E], f16, tag="wh")
            wh_src = wh_ext[:].rearrange("(k p) e -> p k e", p=128)
            wh_dst = wh_sb[:].rearrange("p (k e) -> p k e", k=KC)
            nc.scalar.dma_start(out=wh_dst[:, : KC // 2], in_=wh_src[:, : KC // 2])
            wd_sb = const_pool.tile([128, KC * E], f16, tag="wd")
            wd_src = wd_ext[:].rearrange("(k p) e -> p k e", p=128)
            wd_dst = wd_sb[:].rearrange("p (k e) -> p k e", k=KC)

            def emit_w_rest():
                # wh half 2 is needed by pass A chunk 16, wd by pass C.
                nc.scalar.dma_start(
                    out=wh_dst[:, KC // 2 :], in_=wh_src[:, KC // 2 :]
                )
                nc.scalar.dma_start(out=wd_dst[:, : KC // 2], in_=wd_src[:, : KC // 2])
                nc.scalar.dma_start(out=wd_dst[:, KC // 2 :], in_=wd_src[:, KC // 2 :])

            def emit_tile(t, first):
                # one XBAR transpose: [128t, 8192] -> [p, kk, t], kk<32 = xh
                # chunk kk, kk>=32 = xl chunk kk-32, d = (kk%32)*128 + p
                xt = xt_pool.tile([128, 2 * KC * 128], f16, tag="xt")
                nc.sync.dma_start_transpose(
                    out=xt[:].rearrange("p (k t) -> p k t", k=2 * KC),
                    in_=x2_ext[t * 128 : (t + 1) * 128, :],
                )
                if first:
                    # issued behind tile 0's XBAR so they don't delay it
                    emit_w_rest()

                lga = plg_pool.tile([128, E], f32, tag="lga")
                lgb = plg_pool.tile([128, E], f32, tag="lgb")
                # pass A: xh @ wh -> lga
                for k in range(KC):
                    nc.tensor.matmul(
                        lga[:],
                        lhsT=xt[:, k * 128 : (k + 1) * 128],
                        rhs=wh_sb[:, k * E : (k + 1) * E],
                        start=(k == 0),
                        stop=(k == KC - 1),
                    )
                # pass B: xl @ wh -> lgb;  pass C: xh @ wd -> lgb
                for k in range(KC):
                    nc.tensor.matmul(
                        lgb[:],
                        lhsT=xt[:, (KC + k) * 128 : (KC + k + 1) * 128],
                        rhs=wh_sb[:, k * E : (k + 1) * E],
                        start=(k == 0),
                        stop=False,
                    )
                for k in range(KC):
                    nc.tensor.matmul(
                        lgb[:],
                        lhsT=xt[:, k * 128 : (k + 1) * 128],
                        rhs=wd_sb[:, k * E : (k + 1) * E],
                        start=False,
                        stop=(k == KC - 1),
                    )

                # combine: logits = lga + lgb / 2048 (DVE, one PSUM reader
                # per bank so each bank frees with one sem)
                logits = mid_pool.tile([128, E], f32, tag="logits")
                nc.vector.tensor_scalar(
                    logits[:],
                    lgb[:],
                    1.0 / LOSCALE,
                    None,
                    op0=mybir.AluOpType.mult,
                )
                nc.vector.tensor_add(logits[:], logits[:], lga[:])

                # ---- selection on raw logits ----
                gs = small_pool.tile([128, G], f32, tag="gs")
                nc.vector.tensor_reduce(
                    gs[:],
                    logits[:].rearrange("p (g e) -> p g e", g=G),
                    axis=mybir.AxisListType.X,
                    op=mybir.AluOpType.max,
                )
                gsort = small_pool.tile([128, 8], f32, tag="gsort")
                nc.vector.max(out=gsort[:], in_=gs[:])
                # bias per group: (gs < 4th-largest) * -1e30
                bias8 = small_pool.tile([128, G], f32, tag="bias8")
                nc.vector.tensor_scalar(
                    bias8[:],
                    gs[:],
                    gsort[:, NL - 1 : NL],
                    NEG_BIG,
                    op0=mybir.AluOpType.is_lt,
                    op1=mybir.AluOpType.mult,
                )
                masked = mid_pool.tile([128, E], f32, tag="masked")
                for g in range(G):
                    nc.vector.tensor_scalar_add(
                        masked[:, g * GSZ : (g + 1) * GSZ],
                        logits[:, g * GSZ : (g + 1) * GSZ],
                        bias8[:, g : g + 1],
                    )
                vals8 = small_pool.tile([128, 8], f32, tag="vals8")
                nc.vector.max(out=vals8[:], in_=masked[:])
                idx8 = small_pool.tile([128, 8], mybir.dt.uint32, tag="idx8")
                nc.vector.max_index(out=idx8[:], in_max=vals8[:], in_values=masked[:])

                # ---- weights: e_k / (S + 1e-9 * Z), shifted by M = top value
                negm = small_pool.tile([128, 1], f32, tag="negm")
                nc.vector.tensor_scalar_mul(negm[:], vals8[:, 0:1], -1.0)
                scr = mid_pool.tile([128, E], f32, tag="scr")
                zfull = small_pool.tile([128, 1], f32, tag="zfull")
                nc.scalar.activation(
                    scr[:],
                    logits[:],
                    mybir.ActivationFunctionType.Exp,
                    bias=negm[:],
                    accum_out=zfull[:],
                )
                e8 = small_pool.tile([128, 8], f32, tag="e8")
                s8 = small_pool.tile([128, 1], f32, tag="s8")
                nc.scalar.activation(
                    e8[:],
                    vals8[:],
                    mybir.ActivationFunctionType.Exp,
                    bias=negm[:],
                    accum_out=s8[:],
                )
                den = small_pool.tile([128, 1], f32, tag="den")
                nc.vector.tensor_scalar(
                    den[:],
                    zfull[:],
                    1.0e-9,
                    None,
                    op0=mybir.AluOpType.mult,
                )
                nc.vector.tensor_add(den[:], den[:], s8[:])
                rcp = small_pool.tile([128, 1], f32, tag="rcp")
                nc.vector.reciprocal(rcp[:], den[:])
                w8 = small_pool.tile([128, 8], f32, tag="w8")
                nc.vector.tensor_scalar_mul(w8[:], e8[:], rcp[:])
                i32 = small_pool.tile([128, 8], mybir.dt.int32, tag="i32")
                nc.vector.tensor_copy(out=i32[:], in_=idx8[:])

                nc.gpsimd.dma_start(
                    out=w_out[t * 128 : (t + 1) * 128, :], in_=w8[:]
                )
                nc.gpsimd.dma_start(
                    out=i_out[t * 128 : (t + 1) * 128, :], in_=i32[:]
                )

            if loop_n is None:
                first = True
                for _r in range(repeat):
                    for t in range(NT):
                        emit_tile(t, first)
                        first = False
            else:
                # bench mode: device-side hardware loop around the 16-tile
                # pass; weights loaded once before the loop.
                emit_w_rest()
                with tc.For_i(0, loop_n, 1):
                    for t in range(NT):
                        emit_tile(t, False)
    return nc


def get_program(repeat=None, loop_n=None):
    key = ("nc", repeat if repeat is not None else REPEAT, loop_n)
    if key not in _CACHE:
        nc = _build_program(repeat, loop_n)
        # Bacc defers register allocation + wait-splitting to finalize();
        # the PJRT path serializes the module as-is, so lower it now.
        nc.finalize()
        _CACHE[key] = nc
    return _CACHE[key]


def _split_inputs(x: np.ndarray, weight: np.ndarray):
    x = np.ascontiguousarray(x, dtype=np.float32)
    xh = x.astype(np.float16)
    xl = ((x - xh.astype(np.float32)) * LOSCALE).astype(np.float16)
    x2 = np.empty((x.shape[0], 2 * DIM), dtype=np.float16)
    x2[:, :DIM] = xh
    x2[:, DIM:] = xl
    wt = np.ascontiguousarray(weight.T, dtype=np.float32)  # [DIM, E]
    wh = wt.astype(np.float16)
    wd = ((wt - wh.astype(np.float32)) * LOSCALE).astype(np.float16)
    return x2, wh, wd


def kernel(x: np.ndarray, weight: np.ndarray, repeat=None, **run_kwargs):
    x2, wh, wd = _split_inputs(x, weight)
    nc = get_program(repeat)
    in_maps = [
        {"x2": x2[c * TPC : (c + 1) * TPC], "wh": wh, "wd": wd}
        for c in range(N_CORES)
    ]
    res = run_bass_kernel_spmd(nc, in_maps, list(range(N_CORES)), **run_kwargs)
    weights = np.concatenate([res.results[c]["weights"] for c in range(N_CORES)], axis=0)
    indices = np.concatenate([res.results[c]["indices"] for c in range(N_CORES)], axis=0)
    _CACHE["last_results"] = res
    return weights.astype(np.float32), indices.astype(np.int32)

